# revision 1
# baseline (speedup 1.0000x reference)
"""GRU autoencoder Trainium2 kernel.

Data-parallel over batch: 8 cores x 64 rows. Per core, the recurrence keeps
the hidden state TRANSPOSED in SBUF (hT[klo, 64*khi+b] = h[b, 128*khi+klo])
so it can be the stationary matmul operand directly. Gates are computed as
h @ W.T with fp32r matmuls (M=64, N=512) accumulating in PSUM on top of a
K=1 bias-seed matmul; z/n gates are transposed back via identity matmuls so
the hidden update runs in transposed space. Decoder z-outputs are computed
one step late so their matmuls fill the PE pipe during the elementwise tail.
"""
import os
import sys
import types

import numpy as np

import concourse.bass as bass
import concourse.mybir as mybir
import concourse.tile as tile
from concourse import bass_utils

F32 = mybir.dt.float32
F32R = mybir.dt.float32r
AF = mybir.ActivationFunctionType
OP = mybir.AluOpType

N_CORES = 8
B, T, I, H = 512, 128, 512, 1024
BL = B // N_CORES  # 64


# ---------------------------------------------------------------- fixups
_CTRL_OPCODES = {"Drain", "NoOp", "EventSemaphore", "AllEngineBarrier", "Halt"}


def _split_multi_waits(nc, max_waits=1):
    """This walrus build allows only one sync-wait per instruction; hoist
    excess waits onto preceding NoOps (same engine, so semantics hold)."""
    for f in nc.m.functions:
        for blk in f.blocks:
            insts = blk.instructions
            if not any(
                i.sync_info is not None
                and i.sync_info.on_wait
                and len(i.sync_info.on_wait) > max_waits
                for i in insts
            ):
                continue
            new = []
            for inst in insts:
                si = inst.sync_info
                if si is not None and si.on_wait and len(si.on_wait) > max_waits:
                    waits = list(si.on_wait)
                    extra, keep = waits[:-max_waits], waits[-max_waits:]
                    for cs in range(0, len(extra), max_waits):
                        nop = mybir.InstNoOp(
                            name=nc.get_next_instruction_name(),
                            engine=inst.engine,
                            ins=[],
                            outs=[],
                            sync_info=mybir.SyncInfo(
                                on_wait=extra[cs : cs + max_waits], on_update=[]
                            ),
                        )
                        nc.register_instruction(nop)
                        new.append(nop)
                    si.on_wait = keep
                new.append(inst)
            insts[:] = new


def _install_ntff_hook():
    if "antenv.axon_hooks" in sys.modules:
        return True
    mod = types.ModuleType("antenv.axon_hooks")
    state = {"hook": None}
    mod.set_axon_ntff_profile_hook = lambda h: state.__setitem__("hook", h)
    mod.get_axon_ntff_profile_hook = lambda: state["hook"]
    sys.modules["antenv.axon_hooks"] = mod
    try:
        import antenv

        antenv.axon_hooks = mod
        from trn_agent_boot.trn_boot import _ntff_profile_via_ctypes

        hook = _ntff_profile_via_ctypes("/opt/axon/libaxon_pjrt.so")
        if hook is None:
            return False
        mod.set_axon_ntff_profile_hook(hook)
        return True
    except Exception:
        return False


# ---------------------------------------------------------------- program
def build_nc(n_steps=T):
    nc = bass.Bass("TRN2", target_bir_lowering=False, debug=False, num_devices=N_CORES)

    xT_d = nc.dram_tensor("xT", [n_steps, 4, 128, BL], F32R, kind="ExternalInput").ap()
    wih_d = nc.dram_tensor("wihT", [4, 128, 3 * H], F32R, kind="ExternalInput").ap()
    whh_d = nc.dram_tensor("whhT", [8, 128, 3 * H], F32R, kind="ExternalInput").ap()
    wcb_d = nc.dram_tensor("wcombT", [8, 128, 4 * H], F32R, kind="ExternalInput").ap()
    wz_d = nc.dram_tensor("wzT", [8, 128, I], F32R, kind="ExternalInput").ap()
    br_d = nc.dram_tensor("brows", [128, 2048], F32R, kind="ExternalInput").ap()
    bz_d = nc.dram_tensor("bzrow", [1, I], F32R, kind="ExternalInput").ap()
    on_d = nc.dram_tensor("ones", [128, 64], F32R, kind="ExternalInput").ap()
    id_d = nc.dram_tensor("iden", [64, 64], F32, kind="ExternalInput").ap()
    h0_d = nc.dram_tensor("h0T", [128, 512], F32R, kind="ExternalInput").ap()
    z_d = nc.dram_tensor("z", [BL, n_steps, I], F32, kind="ExternalOutput").ap()

    with tile.TileContext(nc) as tc:
        with (
            tc.tile_pool(name="cst", bufs=1) as cst,
            tc.tile_pool(name="hst", bufs=3) as hst,
            tc.tile_pool(name="xts", bufs=3) as xts,
            tc.tile_pool(name="gsb", bufs=2) as gsb,
            tc.tile_pool(name="tmp", bufs=2) as tmpp,
            tc.tile_pool(name="zo", bufs=2) as zop,
            tc.tile_pool(name="ps", bufs=8, space="PSUM") as ps,
        ):
            brows = cst.tile([128, 2048], F32R)
            nc.sync.dma_start(brows[:], br_d[:])
            bzrow = cst.tile([1, I], F32R)
            nc.sync.dma_start(bzrow[:], bz_d[:])
            ones = cst.tile([128, 64], F32R)
            nc.sync.dma_start(ones[:], on_d[:])
            iden = cst.tile([64, 64], F32)
            nc.sync.dma_start(iden[:], id_d[:])
            hT = hst.tile([128, 512], F32R, tag="h")
            nc.sync.dma_start(hT[:], h0_d[:])

            def seed(pt, brow_ap, one_ap, bp):
                nc.tensor.matmul(pt[:], one_ap, brow_ap, start=True, stop=False,
                                 tile_position=(bp, 0))

            def alloc_seed_pair(nm, t, enc):
                """Allocate+bias-seed one gate pair (2 psum tiles)."""
                bp = 0 if enc else 64
                row, c0 = {
                    "pr": (bp, 0), "pz": (bp, 1024),
                    "pin": (bp + 32, 0), "phn": (bp + 32, 1024),
                }[nm]
                tiles = [ps.tile([64, 512], F32, tag="ps", name=f"{nm}{i}_{t}")
                         for i in range(2)]
                for nt in range(2):
                    seed(tiles[nt], brows[row : row + 1, c0 + 512 * nt : c0 + 512 * nt + 512],
                         ones[row : row + 1, :], row)
                return tiles

            def emit_gi_zr(g, xt):
                for tiles, c0 in ((g["pz"], 1024), (g["pr"], 0)):
                    for nt in range(2):
                        c = c0 + 512 * nt
                        for k in range(4):
                            nc.tensor.matmul(
                                tiles[nt][:], xt[:, k, :], wih[:, k, c : c + 512],
                                start=False, stop=False,
                            )

            def emit_gi_in(g, xt):
                for nt in range(2):
                    c = 2048 + 512 * nt
                    for k in range(4):
                        nc.tensor.matmul(
                            g["pin"][nt][:], xt[:, k, :], wih[:, k, c : c + 512],
                            start=False, stop=(k == 3),
                        )

            def emit_gh(g, w, cols):
                """Recurrent gate matmuls reading hT: order z, r, (in), hn."""
                for nm, c0 in cols:
                    for nt in range(2):
                        c = c0 + 512 * nt
                        for k in range(8):
                            nc.tensor.matmul(
                                g[nm][nt][:],
                                hT[:, 64 * k : 64 * k + 64],
                                w[:, k, c : c + 512],
                                start=False, stop=(k == 7),
                            )

            def emit_zfill(src_hT, t_out):
                pzo = ps.tile([64, 512], F32, tag="ps", name=f"pzo{t_out}")
                nc.tensor.matmul(pzo[:], ones[0:1, :], bzrow[0:1, :],
                                 start=True, stop=False)
                for j in range(8):
                    nc.tensor.matmul(
                        pzo[:], src_hT[:, 64 * j : 64 * j + 64], wz[:, j, :],
                        start=False, stop=(j == 7),
                    )
                zo_sb = zop.tile([64, 512], F32, tag="zo", name=f"zo{t_out}")
                nc.vector.tensor_copy(zo_sb[:], pzo[:])
                nc.sync.dma_start(z_d[:, t_out, :], zo_sb[:])

            def step_tail(t, g, filler):
                """sigmoids, transposes, n-chain, h-update; filler() emits
                next-step PE work between zT and nT transposes."""
                nonlocal hT
                z_sb = gsb.tile([64, 1024], F32, tag="z", name=f"z{t}")
                for nt in range(2):
                    nc.scalar.activation(z_sb[:, 512 * nt : 512 * nt + 512],
                                         g["pz"][nt][:], AF.Sigmoid)
                pzT = ps.tile([128, 512], F32, tag="ps", name=f"pzT{t}")
                for jh in range(8):
                    nc.tensor.matmul(
                        pzT[:, 64 * jh : 64 * jh + 64],
                        z_sb[0:64, 128 * jh : 128 * jh + 128],
                        iden[:, :], start=True, stop=True,
                    )
                r_sb = gsb.tile([64, 1024], F32, tag="r", name=f"r{t}")
                for nt in range(2):
                    nc.scalar.activation(r_sb[:, 512 * nt : 512 * nt + 512],
                                         g["pr"][nt][:], AF.Sigmoid)

                if filler is not None:
                    filler()

                # n = tanh(in + r*hn) per half; transpose blocks as halves land
                n_sb = gsb.tile([64, 1024], F32, tag="n", name=f"n{t}")
                pnT = ps.tile([128, 512], F32, tag="ps", name=f"pnT{t}")
                for nt in range(2):
                    t1 = tmpp.tile([64, 512], F32, tag="t1", name=f"t1_{t}_{nt}")
                    nc.vector.tensor_mul(t1[:], r_sb[:, 512 * nt : 512 * nt + 512],
                                         g["phn"][nt][:])
                    nc.vector.tensor_add(t1[:], t1[:], g["pin"][nt][:])
                    nc.scalar.activation(n_sb[:, 512 * nt : 512 * nt + 512],
                                         t1[:], AF.Tanh)
                    for jh in range(4 * nt, 4 * nt + 4):
                        nc.tensor.matmul(
                            pnT[:, 64 * jh : 64 * jh + 64],
                            n_sb[0:64, 128 * jh : 128 * jh + 128],
                            iden[:, :], start=True, stop=True,
                        )

                # hT' = nT + zT*(hT - nT), pipelined by 256-col halves
                hT_new = hst.tile([128, 512], F32R, tag="h", name=f"h{t}")
                for hh in range(2):
                    s = slice(256 * hh, 256 * hh + 256)
                    d = tmpp.tile([128, 256], F32, tag="d", name=f"d{t}_{hh}")
                    nc.vector.tensor_sub(d[:], hT[:, s], pnT[:, s])
                    nc.vector.tensor_mul(d[:], pzT[:, s], d[:])
                    nc.vector.tensor_add(hT_new[:, s], pnT[:, s], d[:])
                hT = hT_new

            # ================= encoder =================
            with tc.tile_pool(name="wenc", bufs=1) as wenc:
                wih = wenc.tile([128, 4, 3 * H], F32R)
                for k in range(4):
                    nc.sync.dma_start(wih[:, k, :], wih_d[k])
                whh = wenc.tile([128, 8, 3 * H], F32R)
                for k in range(8):
                    nc.sync.dma_start(whh[:, k, :], whh_d[k])
                xt_tiles = {}
                for t in range(min(3, n_steps)):
                    xt_tiles[t] = xts.tile([128, 4, BL], F32R, tag="x", name=f"xt{t}")
                    for k in range(4):
                        nc.sync.dma_start(xt_tiles[t][:, k, :], xT_d[t, k])
                cur = {}
                cur["pz"] = alloc_seed_pair("pz", 0, True)
                cur["pr"] = alloc_seed_pair("pr", 0, True)
                emit_gi_zr(cur, xt_tiles[0])
                cur["pin"] = alloc_seed_pair("pin", 0, True)
                emit_gi_in(cur, xt_tiles[0])
                cur["phn"] = alloc_seed_pair("phn", 0, True)
                for t in range(n_steps):
                    if t + 3 < n_steps:
                        xt_tiles[t + 3] = xts.tile([128, 4, BL], F32R, tag="x",
                                                   name=f"xt{t+3}")
                        for k in range(4):
                            nc.sync.dma_start(xt_tiles[t + 3][:, k, :], xT_d[t + 3, k])
                    emit_gh(cur, whh, (("pz", 1024), ("pr", 0), ("phn", 2048)))
                    g = cur
                    nxt = {}
                    if t + 1 < n_steps:
                        xt_next = xt_tiles[t + 1]

                        def filler(nxt=nxt, xt_next=xt_next, t=t):
                            nxt["pz"] = alloc_seed_pair("pz", t + 1, True)
                            nxt["pr"] = alloc_seed_pair("pr", t + 1, True)
                            emit_gi_zr(nxt, xt_next)
                    else:
                        filler = None
                    step_tail(t, g, filler)
                    if t + 1 < n_steps:
                        nxt["pin"] = alloc_seed_pair("pin", t + 1, True)
                        emit_gi_in(nxt, xt_tiles[t + 1])
                        nxt["phn"] = alloc_seed_pair("phn", t + 1, True)
                    cur = nxt
                    xt_tiles.pop(t, None)

            # ================= decoder =================
            with tc.tile_pool(name="wdec", bufs=1) as wdec:
                wcb = wdec.tile([128, 8, 4 * H], F32R)
                for k in range(8):
                    nc.sync.dma_start(wcb[:, k, :], wcb_d[k])
                wz = wdec.tile([128, 8, I], F32R)
                for k in range(8):
                    nc.sync.dma_start(wz[:, k, :], wz_d[k])
                cur = {}
                for nm in ("pz", "pr", "pin", "phn"):
                    cur[nm] = alloc_seed_pair(nm, 1000, False)
                for t in range(n_steps):
                    emit_gh(cur, wcb, (("pz", 1024), ("pr", 0), ("pin", 2048),
                                       ("phn", 3072)))
                    g = cur
                    hT_entry = hT
                    nxt = {}
                    if t + 1 < n_steps:

                        def filler(nxt=nxt, hT_entry=hT_entry, t=t):
                            nxt["pz"] = alloc_seed_pair("pz", 1001 + t, False)
                            nxt["pr"] = alloc_seed_pair("pr", 1001 + t, False)
                            if t >= 1:
                                emit_zfill(hT_entry, t - 1)
                    else:

                        def filler(hT_entry=hT_entry, t=t):
                            emit_zfill(hT_entry, t - 1)
                    step_tail(1000 + t, g, filler)
                    if t + 1 < n_steps:
                        nxt["pin"] = alloc_seed_pair("pin", 1001 + t, False)
                        nxt["phn"] = alloc_seed_pair("phn", 1001 + t, False)
                    cur = nxt
                # final z output from last hidden state
                emit_zfill(hT, n_steps - 1)
    return nc


# ---------------------------------------------------------------- host side
def _prep_shared(enc_Wih, enc_Whh, enc_bih, enc_bhh,
                 dec_Wih, dec_Whh, dec_bih, dec_bhh, Wz, bz):
    f32 = np.float32
    wihT = np.ascontiguousarray(enc_Wih.T, dtype=f32).reshape(4, 128, 3 * H)
    whhT = np.ascontiguousarray(enc_Whh.T, dtype=f32).reshape(8, 128, 3 * H)
    wcomb = np.concatenate(
        [dec_Wih[: 2 * H] + dec_Whh[: 2 * H], dec_Wih[2 * H :], dec_Whh[2 * H :]], 0
    )
    wcombT = np.ascontiguousarray(wcomb.T, dtype=f32).reshape(8, 128, 4 * H)
    wzT = np.ascontiguousarray(Wz.T, dtype=f32).reshape(8, 128, I)
    brows = np.zeros((128, 2048), f32)
    brows[0] = (enc_bih + enc_bhh)[: 2 * H]
    brows[32] = np.concatenate([enc_bih[2 * H :], enc_bhh[2 * H :]])
    brows[64] = (dec_bih + dec_bhh)[: 2 * H]
    brows[96] = np.concatenate([dec_bih[2 * H :], dec_bhh[2 * H :]])
    ones = np.ones((128, 64), f32)
    iden = np.eye(64, dtype=f32)
    h0T = np.full((128, 512), 0.1, f32)
    return {
        "wihT": wihT, "whhT": whhT, "wcombT": wcombT, "wzT": wzT,
        "brows": brows, "bzrow": np.asarray(bz, f32)[None, :],
        "ones": ones, "iden": iden, "h0T": h0T,
    }


def kernel(x, enc_Wih, enc_Whh, enc_bih, enc_bhh,
           dec_Wih, dec_Whh, dec_bih, dec_bhh, Wz, bz, n_steps=T):
    x = np.asarray(x, np.float32)
    shared = _prep_shared(enc_Wih, enc_Whh, enc_bih, enc_bhh,
                          dec_Wih, dec_Whh, dec_bih, dec_bhh, Wz, bz)
    in_maps = []
    for c in range(N_CORES):
        xc = x[c * BL : (c + 1) * BL, :n_steps]  # [BL, n_steps, I]
        xT = np.ascontiguousarray(xc.transpose(1, 2, 0)).reshape(n_steps, 4, 128, BL)
        in_maps.append({"xT": xT, **shared})

    nc = build_nc(n_steps)
    _split_multi_waits(nc)

    trace = bool(int(os.environ.get("GRU_TRACE", "0")))
    if trace:
        _install_ntff_hook()
    res = bass_utils.run_bass_kernel_spmd(
        nc, in_maps, core_ids=list(range(N_CORES)), trace=trace
    )
    if trace and res.exec_time_ns is not None:
        print(f"HW exec time: {res.exec_time_ns} ns")
    out = np.concatenate([res.results[c]["z"] for c in range(N_CORES)], axis=0)
    return out



# revision 10
# speedup vs baseline: 1.3797x; 1.3797x over previous
"""GRU autoencoder Trainium2 kernel.

Data-parallel over batch: 8 cores x 64 rows. Per core, the recurrence keeps
the hidden state TRANSPOSED in SBUF (hT[klo, 64*khi+b] = h[b, 128*khi+klo])
so it can be the stationary matmul operand directly. Gates are computed as
h @ W.T with fp16 weights (moving operand) accumulating in PSUM; PSUM bias
init is done by Act/DVE copies from precomputed bias tiles instead of K=1
matmul seeds, keeping the PE free for gate matmuls. z/n gates are transposed
back via identity matmuls so the hidden update runs in transposed space.
Decoder z-outputs are computed in step-pairs (M=128 stationary spanning a
double-wide hT tile), with the 8-matmul chain split across two consecutive
step tails so the PE pipe stays full.
"""
import os
import sys
import types

import numpy as np

import concourse.bass as bass
import concourse.mybir as mybir
import concourse.tile as tile
from concourse import bass_utils

F32 = mybir.dt.float32
F16 = mybir.dt.float16
AF = mybir.ActivationFunctionType
OP = mybir.AluOpType

N_CORES = 8
B, T, I, H = 512, 128, 512, 1024
BL = B // N_CORES  # 64


# ---------------------------------------------------------------- fixups
_CTRL_OPCODES = {"Drain", "NoOp", "EventSemaphore", "AllEngineBarrier", "Halt"}


def _split_multi_waits(nc, max_waits=1):
    """This walrus build allows only one sync-wait per instruction; hoist
    excess waits onto preceding NoOps (same engine, so semantics hold)."""
    for f in nc.m.functions:
        for blk in f.blocks:
            insts = blk.instructions
            if not any(
                i.sync_info is not None
                and i.sync_info.on_wait
                and len(i.sync_info.on_wait) > max_waits
                for i in insts
            ):
                continue
            new = []
            for inst in insts:
                si = inst.sync_info
                if si is not None and si.on_wait and len(si.on_wait) > max_waits:
                    waits = list(si.on_wait)
                    extra, keep = waits[:-max_waits], waits[-max_waits:]
                    for cs in range(0, len(extra), max_waits):
                        nop = mybir.InstNoOp(
                            name=nc.get_next_instruction_name(),
                            engine=inst.engine,
                            ins=[],
                            outs=[],
                            sync_info=mybir.SyncInfo(
                                on_wait=extra[cs : cs + max_waits], on_update=[]
                            ),
                        )
                        nc.register_instruction(nop)
                        new.append(nop)
                    si.on_wait = keep
                new.append(inst)
            insts[:] = new


def _install_ntff_hook():
    if "antenv.axon_hooks" in sys.modules:
        return True
    mod = types.ModuleType("antenv.axon_hooks")
    state = {"hook": None}
    mod.set_axon_ntff_profile_hook = lambda h: state.__setitem__("hook", h)
    mod.get_axon_ntff_profile_hook = lambda: state["hook"]
    sys.modules["antenv.axon_hooks"] = mod
    try:
        import antenv

        antenv.axon_hooks = mod
        from trn_agent_boot.trn_boot import _ntff_profile_via_ctypes

        hook = _ntff_profile_via_ctypes("/opt/axon/libaxon_pjrt.so")
        if hook is None:
            return False
        mod.set_axon_ntff_profile_hook(hook)
        return True
    except Exception:
        return False


# ---------------------------------------------------------------- program
def build_nc(n_steps=T):
    nc = bass.Bass("TRN2", target_bir_lowering=False, debug=False, num_devices=N_CORES)

    xT_d = nc.dram_tensor("xT", [n_steps, 4, 128, BL], F16, kind="ExternalInput").ap()
    wih_d = nc.dram_tensor("wihT", [4, 128, 3 * H], F16, kind="ExternalInput").ap()
    whh_d = nc.dram_tensor("whhT", [8, 128, 3 * H], F16, kind="ExternalInput").ap()
    wcb_d = nc.dram_tensor("wcombT", [8, 128, 4 * H], F16, kind="ExternalInput").ap()
    wz_d = nc.dram_tensor("wzT", [8, 128, I], F16, kind="ExternalInput").ap()
    be_d = nc.dram_tensor("biasE", [8, 64, 512], F16, kind="ExternalInput").ap()
    bd_d = nc.dram_tensor("biasD", [8, 64, 512], F16, kind="ExternalInput").ap()
    bzb_d = nc.dram_tensor("bzb", [128, I], F32, kind="ExternalInput").ap()
    id_d = nc.dram_tensor("iden", [64, 64], F16, kind="ExternalInput").ap()
    h0_d = nc.dram_tensor("h0T", [128, 512], F16, kind="ExternalInput").ap()
    z_d = nc.dram_tensor("z", [BL, n_steps, I], F32, kind="ExternalOutput").ap()

    with tile.TileContext(nc) as tc:
        with (
            tc.tile_pool(name="cst", bufs=1) as cst,
            tc.tile_pool(name="hst", bufs=3) as hst,
            tc.tile_pool(name="hsd", bufs=3) as hsd,
            tc.tile_pool(name="xts", bufs=3) as xts,
            tc.tile_pool(name="gsb", bufs=2) as gsb,
            tc.tile_pool(name="tmp", bufs=2) as tmpp,
            tc.tile_pool(name="zo", bufs=2) as zop,
            tc.tile_pool(name="ps", bufs=8, space="PSUM") as ps,
        ):
            biasE = cst.tile([64, 8, 512], F16)
            for j in range(8):
                nc.sync.dma_start(biasE[:, j, :], be_d[j])
            biasD = cst.tile([64, 8, 512], F16)
            for j in range(8):
                nc.sync.dma_start(biasD[:, j, :], bd_d[j])
            bzb = cst.tile([128, I], F32)
            nc.sync.dma_start(bzb[:], bzb_d[:])
            iden = cst.tile([64, 64], F16)
            nc.sync.dma_start(iden[:], id_d[:])
            hT0 = hst.tile([128, 512], F16, tag="h")
            nc.sync.dma_start(hT0[:], h0_d[:])
            # all weights resident in fp16 (18MB total)
            wih = cst.tile([128, 4, 3 * H], F16)
            for k in range(4):
                nc.sync.dma_start(wih[:, k, :], wih_d[k])
            whh = cst.tile([128, 8, 3 * H], F16)
            for k in range(8):
                nc.sync.dma_start(whh[:, k, :], whh_d[k])
            wcb = cst.tile([128, 8, 4 * H], F16)
            for k in range(8):
                nc.sync.dma_start(wcb[:, k, :], wcb_d[k])
            wz = cst.tile([128, 8, I], F16)
            for k in range(8):
                nc.sync.dma_start(wz[:, k, :], wz_d[k])

            BIAS_J = {"pz": 0, "pr": 2, "pin": 4, "phn": 6}

            def alloc_pair(nm, t):
                """Allocate one gate pair (2 psum tiles)."""
                return [ps.tile([64, 512], F32, tag="ps", name=f"{nm}{i}_{t}")
                        for i in range(2)]

            def emit_gi_zr(g, xt):
                for tiles, c0 in ((g["pz"], 1024), (g["pr"], 0)):
                    for nt in range(2):
                        c = c0 + 512 * nt
                        for k in range(4):
                            nc.tensor.matmul(
                                tiles[nt][:], xt[:, k, :], wih[:, k, c : c + 512],
                                start=(k == 0), stop=False, skip_group_check=True,
                            )

            def emit_gi_in(g, xt):
                for nt in range(2):
                    c = 2048 + 512 * nt
                    for k in range(4):
                        nc.tensor.matmul(
                            g["pin"][nt][:], xt[:, k, :], wih[:, k, c : c + 512],
                            start=(k == 0), stop=(k == 3), skip_group_check=True,
                        )

            def emit_gh(g, w, cols, stat, starts=()):
                """Recurrent gate matmuls; stat(k) -> [128,64] stationary AP."""
                for nm, c0 in cols:
                    for nt in range(2):
                        c = c0 + 512 * nt
                        for k in range(8):
                            nc.tensor.matmul(
                                g[nm][nt][:], stat(k), w[:, k, c : c + 512],
                                start=(k == 0 and nm in starts),
                                stop=(k == 7), skip_group_check=True,
                            )

            def zfill_first(hTd_pair, t0):
                """z-output pair (t0, t0+1): bias init + first 4 k-chunks."""
                pzo = ps.tile([128, 512], F32, tag="ps", name=f"pzo{t0}")
                for j in range(4):
                    nc.tensor.matmul(
                        pzo[:], hTd_pair[:, j, :, :], wz[:, j, :],
                        start=(j == 0), stop=False, skip_group_check=True,
                    )
                return pzo

            def zfill_second(pzo, hTd_pair, t0):
                """z-output pair (t0, t0+1): last 4 k-chunks + writeback."""
                for j in range(4, 8):
                    nc.tensor.matmul(
                        pzo[:], hTd_pair[:, j, :, :], wz[:, j, :],
                        start=False, stop=(j == 7), skip_group_check=True,
                    )
                zo_sb = zop.tile([128, 512], F32, tag="zo", name=f"zo{t0}")
                nc.vector.tensor_add(zo_sb[:], pzo[:], bzb[:])
                nc.sync.dma_start(z_d[:, t0, :], zo_sb[0:64, :])
                nc.sync.dma_start(z_d[:, t0 + 1, :], zo_sb[64:128, :])

            def step_tail(t, g, filler, hin, hout, bias):
                """sigmoids, transposes, n-chain, h-update; filler() emits
                next-step PE work between zT and nT transposes.
                hin(hh)/hout(hh) -> [128, 256]-sized APs for half hh."""
                z_sb = gsb.tile([64, 1024], F16, tag="z", name=f"z{t}")
                for nt in range(2):
                    nc.vector.tensor_add(g["pz"][nt][:], g["pz"][nt][:],
                                         bias[:, 0 + nt, :])
                    nc.scalar.activation(z_sb[:, 512 * nt : 512 * nt + 512],
                                         g["pz"][nt][:], AF.Sigmoid)
                pzT = ps.tile([128, 512], F32, tag="ps", name=f"pzT{t}")
                for jh in range(8):
                    nc.tensor.matmul(
                        pzT[:, 64 * jh : 64 * jh + 64],
                        z_sb[0:64, 128 * jh : 128 * jh + 128],
                        iden[:, :], start=True, stop=True,
                    )
                r_sb = gsb.tile([64, 1024], F16, tag="r", name=f"r{t}")
                for nt in range(2):
                    nc.vector.tensor_add(g["pr"][nt][:], g["pr"][nt][:],
                                         bias[:, 2 + nt, :])
                    nc.scalar.activation(r_sb[:, 512 * nt : 512 * nt + 512],
                                         g["pr"][nt][:], AF.Sigmoid)

                if filler is not None:
                    filler()

                # n = tanh(in + r*hn) per half; transpose blocks as halves land
                n_sb = gsb.tile([64, 1024], F16, tag="n", name=f"n{t}")
                pnT = ps.tile([128, 512], F32, tag="ps", name=f"pnT{t}")
                for nt in range(2):
                    t1 = tmpp.tile([64, 512], F32, tag="t1", name=f"t1_{t}_{nt}")
                    nc.vector.tensor_add(g["phn"][nt][:], g["phn"][nt][:],
                                         bias[:, 6 + nt, :])
                    nc.vector.tensor_mul(t1[:], r_sb[:, 512 * nt : 512 * nt + 512],
                                         g["phn"][nt][:])
                    nc.vector.tensor_add(g["pin"][nt][:], g["pin"][nt][:],
                                         bias[:, 4 + nt, :])
                    nc.vector.tensor_add(t1[:], t1[:], g["pin"][nt][:])
                    nc.scalar.activation(n_sb[:, 512 * nt : 512 * nt + 512],
                                         t1[:], AF.Tanh)
                    for jh in range(4 * nt, 4 * nt + 4):
                        nc.tensor.matmul(
                            pnT[:, 64 * jh : 64 * jh + 64],
                            n_sb[0:64, 128 * jh : 128 * jh + 128],
                            iden[:, :], start=True, stop=True,
                        )

                # hT' = nT + zT*(hT - nT), pipelined by 256-col halves
                for hh in range(2):
                    s = slice(256 * hh, 256 * hh + 256)
                    d = tmpp.tile([128, 256], F32, tag="d", name=f"d{t}_{hh}")
                    nc.vector.tensor_sub(d[:], hin(hh), pnT[:, s])
                    nc.vector.tensor_mul(d[:], pzT[:, s], d[:])
                    nc.vector.tensor_add(hout(hh), pnT[:, s], d[:])

            # ================= encoder =================
            xt_tiles = {}
            for t in range(min(3, n_steps)):
                xt_tiles[t] = xts.tile([128, 4, BL], F16, tag="x", name=f"xt{t}")
                for k in range(4):
                    nc.sync.dma_start(xt_tiles[t][:, k, :], xT_d[t, k])
            cur = {}
            cur["pz"] = alloc_pair("pz", 0)
            cur["pr"] = alloc_pair("pr", 0)
            emit_gi_zr(cur, xt_tiles[0])
            cur["pin"] = alloc_pair("pin", 0)
            emit_gi_in(cur, xt_tiles[0])
            cur["phn"] = alloc_pair("phn", 0)
            hT = hT0
            for t in range(n_steps):
                if t + 3 < n_steps:
                    xt_tiles[t + 3] = xts.tile([128, 4, BL], F16, tag="x",
                                               name=f"xt{t+3}")
                    for k in range(4):
                        nc.sync.dma_start(xt_tiles[t + 3][:, k, :], xT_d[t + 3, k])
                hT_prev = hT
                emit_gh(cur, whh, (("pz", 1024), ("pr", 0), ("phn", 2048)),
                        lambda k: hT_prev[:, 64 * k : 64 * k + 64],
                        starts=("phn",))
                g = cur
                nxt = {}
                if t + 1 < n_steps:
                    xt_next = xt_tiles[t + 1]

                    def filler(nxt=nxt, xt_next=xt_next, t=t):
                        nxt["pz"] = alloc_pair("pz", t + 1)
                        nxt["pr"] = alloc_pair("pr", t + 1)
                        emit_gi_zr(nxt, xt_next)
                else:
                    filler = None
                hT_new = hst.tile([128, 512], F16, tag="h", name=f"h{t}")
                step_tail(
                    t, g, filler,
                    lambda hh, hT_prev=hT_prev: hT_prev[:, 256 * hh : 256 * hh + 256],
                    lambda hh, hT_new=hT_new: hT_new[:, 256 * hh : 256 * hh + 256],
                    biasE,
                )
                hT = hT_new
                if t + 1 < n_steps:
                    nxt["pin"] = alloc_pair("pin", t + 1)
                    emit_gi_in(nxt, xt_tiles[t + 1])
                    nxt["phn"] = alloc_pair("phn", t + 1)
                cur = nxt
                xt_tiles.pop(t, None)

            # ================= decoder =================
            enc_hT = hT  # [128, 512] final encoder state
            cur = {}
            for nm in ("pz", "pr", "pin", "phn"):
                cur[nm] = alloc_pair(nm, 1000)
            hTd = None       # current pair tile [128, 2, 8, BL]
            hTd_prev = None  # previous pair tile
            pend = None      # (pzo, pair_tile, t0) with k4..7 outstanding
            for t in range(n_steps):
                if t == 0:
                    stat = lambda k: enc_hT[:, 64 * k : 64 * k + 64]
                elif t % 2 == 1:
                    stat = lambda k, _h=hTd: _h[:, k, 0, :]
                else:
                    stat = lambda k, _h=hTd: _h[:, k, 1, :]
                if t % 2 == 0:
                    hTd_prev = hTd
                    hTd = hsd.tile([128, 8, 2, BL], F16, tag="hd", name=f"hd{t}")
                emit_gh(cur, wcb, (("pz", 1024), ("pr", 0), ("pin", 2048),
                                   ("phn", 3072)), stat,
                        starts=("pz", "pr", "pin", "phn"))
                g = cur
                nxt = {}

                def filler(nxt=nxt, t=t, zp=hTd_prev, last=(t + 1 >= n_steps)):
                    nonlocal pend
                    if t % 2 == 0 and pend is not None:
                        pzo, zp2, t0 = pend
                        zfill_second(pzo, zp2, t0)
                        pend = None
                    if not last:
                        nxt["pz"] = alloc_pair("pz", 1001 + t)
                        nxt["pr"] = alloc_pair("pr", 1001 + t)
                    if t % 2 == 1 and t >= 3:
                        pend = (zfill_first(zp, t - 3), zp, t - 3)

                hin_t = (
                    (lambda hh, _e=enc_hT: _e[:, 256 * hh : 256 * hh + 256])
                    if t == 0
                    else (lambda hh, _h=hTd if t % 2 == 1 else hTd_prev,
                          _half=(t - 1) % 2:
                          _h[:, 4 * hh : 4 * hh + 4, _half, :])
                )
                hout_t = (lambda hh, _h=hTd, _half=t % 2:
                          _h[:, 4 * hh : 4 * hh + 4, _half, :])
                step_tail(1000 + t, g, filler, hin_t, hout_t, biasD)
                if t + 1 < n_steps:
                    nxt["pin"] = alloc_pair("pin", 1001 + t)
                    nxt["phn"] = alloc_pair("phn", 1001 + t)
                cur = nxt
            # flush remaining z pairs: (124,125) second half, then (126,127)
            if pend is not None:
                pzo, zp2, t0 = pend
                zfill_second(pzo, zp2, t0)
            pzo = zfill_first(hTd, n_steps - 2)
            zfill_second(pzo, hTd, n_steps - 2)
    return nc


# ---------------------------------------------------------------- host side
def _prep_shared(enc_Wih, enc_Whh, enc_bih, enc_bhh,
                 dec_Wih, dec_Whh, dec_bih, dec_bhh, Wz, bz):
    f16, f32 = np.float16, np.float32
    wihT = np.ascontiguousarray(enc_Wih.T, dtype=f16).reshape(4, 128, 3 * H)
    whhT = np.ascontiguousarray(enc_Whh.T, dtype=f16).reshape(8, 128, 3 * H)
    wcomb = np.concatenate(
        [dec_Wih[: 2 * H] + dec_Whh[: 2 * H], dec_Wih[2 * H :], dec_Whh[2 * H :]], 0
    )
    wcombT = np.ascontiguousarray(wcomb.T, dtype=f16).reshape(8, 128, 4 * H)
    wzT = np.ascontiguousarray(Wz.T, dtype=f16).reshape(8, 128, I)

    def bias8(bih, bhh):
        rz = np.asarray(bih[: 2 * H] + bhh[: 2 * H], f32)
        rows = np.stack([
            rz[1024:1536], rz[1536:2048],          # z0, z1
            rz[0:512], rz[512:1024],               # r0, r1
            np.asarray(bih[2048:2560], f32), np.asarray(bih[2560:3072], f32),
            np.asarray(bhh[2048:2560], f32), np.asarray(bhh[2560:3072], f32),
        ])  # [8, 512]
        return np.ascontiguousarray(
            np.broadcast_to(rows[:, None, :], (8, 64, 512)), dtype=f16)

    biasE = bias8(enc_bih, enc_bhh)
    biasD = bias8(dec_bih, dec_bhh)
    bzb = np.ascontiguousarray(
        np.broadcast_to(np.asarray(bz, f32)[None, :], (128, I)), dtype=f32)
    iden = np.eye(64, dtype=f16)
    h0T = np.full((128, 512), 0.1, f16)
    return {
        "wihT": wihT, "whhT": whhT, "wcombT": wcombT, "wzT": wzT,
        "biasE": biasE, "biasD": biasD, "bzb": bzb,
        "iden": iden, "h0T": h0T,
    }


def kernel(x, enc_Wih, enc_Whh, enc_bih, enc_bhh,
           dec_Wih, dec_Whh, dec_bih, dec_bhh, Wz, bz, n_steps=T):
    x = np.asarray(x, np.float32)
    shared = _prep_shared(enc_Wih, enc_Whh, enc_bih, enc_bhh,
                          dec_Wih, dec_Whh, dec_bih, dec_bhh, Wz, bz)
    in_maps = []
    for c in range(N_CORES):
        xc = x[c * BL : (c + 1) * BL, :n_steps]  # [BL, n_steps, I]
        xT = np.ascontiguousarray(
            xc.transpose(1, 2, 0), dtype=np.float16).reshape(n_steps, 4, 128, BL)
        in_maps.append({"xT": xT, **shared})

    nc = build_nc(n_steps)
    _split_multi_waits(nc)

    trace = bool(int(os.environ.get("GRU_TRACE", "0")))
    if trace:
        _install_ntff_hook()
    res = bass_utils.run_bass_kernel_spmd(
        nc, in_maps, core_ids=list(range(N_CORES)), trace=trace
    )
    if trace and res.exec_time_ns is not None:
        print(f"HW exec time: {res.exec_time_ns} ns")
    out = np.concatenate([res.results[c]["z"] for c in range(N_CORES)], axis=0)
    return out


# revision 11
# speedup vs baseline: 1.3798x; 1.0001x over previous
"""GRU autoencoder Trainium2 kernel.

Data-parallel over batch: 8 cores x 64 rows. Per core, the recurrence keeps
the hidden state TRANSPOSED in SBUF (hT[klo, 64*khi+b] = h[b, 128*khi+klo])
so it can be the stationary matmul operand directly. Gates are computed as
h @ W.T with fp16 weights (moving operand) accumulating in PSUM; PSUM bias
init is done by Act/DVE copies from precomputed bias tiles instead of K=1
matmul seeds, keeping the PE free for gate matmuls. z/n gates are transposed
back via identity matmuls so the hidden update runs in transposed space.
Decoder z-outputs are computed in step-pairs (M=128 stationary spanning a
double-wide hT tile), with the 8-matmul chain split across two consecutive
step tails so the PE pipe stays full.
"""
import os
import sys
import types

import numpy as np

import concourse.bass as bass
import concourse.mybir as mybir
import concourse.tile as tile
from concourse import bass_utils

F32 = mybir.dt.float32
F16 = mybir.dt.float16
AF = mybir.ActivationFunctionType
OP = mybir.AluOpType

N_CORES = 8
B, T, I, H = 512, 128, 512, 1024
BL = B // N_CORES  # 64


# ---------------------------------------------------------------- fixups
_CTRL_OPCODES = {"Drain", "NoOp", "EventSemaphore", "AllEngineBarrier", "Halt"}


def _split_multi_waits(nc, max_waits=1):
    """This walrus build allows only one sync-wait per instruction; hoist
    excess waits onto preceding NoOps (same engine, so semantics hold)."""
    for f in nc.m.functions:
        for blk in f.blocks:
            insts = blk.instructions
            if not any(
                i.sync_info is not None
                and i.sync_info.on_wait
                and len(i.sync_info.on_wait) > max_waits
                for i in insts
            ):
                continue
            new = []
            for inst in insts:
                si = inst.sync_info
                if si is not None and si.on_wait and len(si.on_wait) > max_waits:
                    waits = list(si.on_wait)
                    extra, keep = waits[:-max_waits], waits[-max_waits:]
                    for cs in range(0, len(extra), max_waits):
                        nop = mybir.InstNoOp(
                            name=nc.get_next_instruction_name(),
                            engine=inst.engine,
                            ins=[],
                            outs=[],
                            sync_info=mybir.SyncInfo(
                                on_wait=extra[cs : cs + max_waits], on_update=[]
                            ),
                        )
                        nc.register_instruction(nop)
                        new.append(nop)
                    si.on_wait = keep
                new.append(inst)
            insts[:] = new


def _install_ntff_hook():
    if "antenv.axon_hooks" in sys.modules:
        return True
    mod = types.ModuleType("antenv.axon_hooks")
    state = {"hook": None}
    mod.set_axon_ntff_profile_hook = lambda h: state.__setitem__("hook", h)
    mod.get_axon_ntff_profile_hook = lambda: state["hook"]
    sys.modules["antenv.axon_hooks"] = mod
    try:
        import antenv

        antenv.axon_hooks = mod
        from trn_agent_boot.trn_boot import _ntff_profile_via_ctypes

        hook = _ntff_profile_via_ctypes("/opt/axon/libaxon_pjrt.so")
        if hook is None:
            return False
        mod.set_axon_ntff_profile_hook(hook)
        return True
    except Exception:
        return False


# ---------------------------------------------------------------- program
def build_nc(n_steps=T):
    nc = bass.Bass("TRN2", target_bir_lowering=False, debug=False, num_devices=N_CORES)

    xT_d = nc.dram_tensor("xT", [n_steps, 4, 128, BL], F16, kind="ExternalInput").ap()
    wih_d = nc.dram_tensor("wihT", [4, 128, 3 * H], F16, kind="ExternalInput").ap()
    whh_d = nc.dram_tensor("whhT", [8, 128, 3 * H], F16, kind="ExternalInput").ap()
    wcb_d = nc.dram_tensor("wcombT", [8, 128, 4 * H], F16, kind="ExternalInput").ap()
    wz_d = nc.dram_tensor("wzT", [8, 128, I], F16, kind="ExternalInput").ap()
    be_d = nc.dram_tensor("biasE", [8, 64, 512], F16, kind="ExternalInput").ap()
    bd_d = nc.dram_tensor("biasD", [8, 64, 512], F16, kind="ExternalInput").ap()
    bzb_d = nc.dram_tensor("bzb", [128, I], F32, kind="ExternalInput").ap()
    id_d = nc.dram_tensor("iden", [64, 64], F16, kind="ExternalInput").ap()
    h0_d = nc.dram_tensor("h0T", [128, 512], F16, kind="ExternalInput").ap()
    z_d = nc.dram_tensor("z", [BL, n_steps, I], F32, kind="ExternalOutput").ap()

    with tile.TileContext(nc) as tc:
        with (
            tc.tile_pool(name="cst", bufs=1) as cst,
            tc.tile_pool(name="hst", bufs=3) as hst,
            tc.tile_pool(name="hsd", bufs=3) as hsd,
            tc.tile_pool(name="xts", bufs=3) as xts,
            tc.tile_pool(name="gsb", bufs=2) as gsb,
            tc.tile_pool(name="tmp", bufs=2) as tmpp,
            tc.tile_pool(name="zo", bufs=2) as zop,
            tc.tile_pool(name="ps", bufs=8, space="PSUM") as ps,
        ):
            biasE = cst.tile([64, 8, 512], F16)
            for j in range(8):
                nc.sync.dma_start(biasE[:, j, :], be_d[j])
            biasD = cst.tile([64, 8, 512], F16)
            for j in range(8):
                nc.sync.dma_start(biasD[:, j, :], bd_d[j])
            bzb = cst.tile([128, I], F32)
            nc.sync.dma_start(bzb[:], bzb_d[:])
            iden = cst.tile([64, 64], F16)
            nc.sync.dma_start(iden[:], id_d[:])
            hT0 = hst.tile([128, 512], F16, tag="h")
            nc.sync.dma_start(hT0[:], h0_d[:])
            # all weights resident in fp16 (18MB total)
            wih = cst.tile([128, 4, 3 * H], F16)
            for k in range(4):
                nc.sync.dma_start(wih[:, k, :], wih_d[k])
            whh = cst.tile([128, 8, 3 * H], F16)
            for k in range(8):
                nc.sync.dma_start(whh[:, k, :], whh_d[k])
            wcb = cst.tile([128, 8, 4 * H], F16)
            for k in range(8):
                nc.sync.dma_start(wcb[:, k, :], wcb_d[k])
            wz = cst.tile([128, 8, I], F16)
            for k in range(8):
                nc.sync.dma_start(wz[:, k, :], wz_d[k])

            BIAS_J = {"pz": 0, "pr": 2, "pin": 4, "phn": 6}

            def alloc_pair(nm, t):
                """Allocate one gate pair (2 psum tiles)."""
                return [ps.tile([64, 512], F32, tag="ps", name=f"{nm}{i}_{t}")
                        for i in range(2)]

            def add_bias(g, bias, names):
                for nm in names:
                    j0 = BIAS_J[nm]
                    for nt in range(2):
                        nc.vector.tensor_add(g[nm][nt][:], g[nm][nt][:],
                                             bias[:, j0 + nt, :])

            def emit_gi_zr(g, xt):
                for tiles, c0 in ((g["pz"], 1024), (g["pr"], 0)):
                    for nt in range(2):
                        c = c0 + 512 * nt
                        for k in range(4):
                            nc.tensor.matmul(
                                tiles[nt][:], xt[:, k, :], wih[:, k, c : c + 512],
                                start=(k == 0), stop=False, skip_group_check=True,
                            )

            def emit_gi_in(g, xt):
                for nt in range(2):
                    c = 2048 + 512 * nt
                    for k in range(4):
                        nc.tensor.matmul(
                            g["pin"][nt][:], xt[:, k, :], wih[:, k, c : c + 512],
                            start=(k == 0), stop=(k == 3), skip_group_check=True,
                        )

            def emit_gh(g, w, cols, stat, starts=()):
                """Recurrent gate matmuls; stat(k) -> [128,64] stationary AP."""
                for nm, c0 in cols:
                    for nt in range(2):
                        c = c0 + 512 * nt
                        for k in range(8):
                            nc.tensor.matmul(
                                g[nm][nt][:], stat(k), w[:, k, c : c + 512],
                                start=(k == 0 and nm in starts),
                                stop=(k == 7), skip_group_check=True,
                            )

            def zfill_first(hTd_pair, t0):
                """z-output pair (t0, t0+1): bias init + first 4 k-chunks."""
                pzo = ps.tile([128, 512], F32, tag="ps", name=f"pzo{t0}")
                for j in range(4):
                    nc.tensor.matmul(
                        pzo[:], hTd_pair[:, j, :, :], wz[:, j, :],
                        start=(j == 0), stop=False, skip_group_check=True,
                    )
                return pzo

            def zfill_second(pzo, hTd_pair, t0):
                """z-output pair (t0, t0+1): last 4 k-chunks + writeback."""
                for j in range(4, 8):
                    nc.tensor.matmul(
                        pzo[:], hTd_pair[:, j, :, :], wz[:, j, :],
                        start=False, stop=(j == 7), skip_group_check=True,
                    )
                zo_sb = zop.tile([128, 512], F32, tag="zo", name=f"zo{t0}")
                nc.vector.tensor_add(zo_sb[:], pzo[:], bzb[:])
                nc.sync.dma_start(z_d[:, t0, :], zo_sb[0:64, :])
                nc.sync.dma_start(z_d[:, t0 + 1, :], zo_sb[64:128, :])

            def step_tail(t, g, filler, hin, hout):
                """sigmoids, transposes, n-chain, h-update; filler() emits
                next-step PE work between zT and nT transposes.
                hin(hh)/hout(hh) -> [128, 256]-sized APs for half hh."""
                z_sb = gsb.tile([64, 1024], F16, tag="z", name=f"z{t}")
                for nt in range(2):
                    nc.scalar.activation(z_sb[:, 512 * nt : 512 * nt + 512],
                                         g["pz"][nt][:], AF.Sigmoid)
                pzT = ps.tile([128, 512], F32, tag="ps", name=f"pzT{t}")
                for jh in range(8):
                    nc.tensor.matmul(
                        pzT[:, 64 * jh : 64 * jh + 64],
                        z_sb[0:64, 128 * jh : 128 * jh + 128],
                        iden[:, :], start=True, stop=True,
                    )
                r_sb = gsb.tile([64, 1024], F16, tag="r", name=f"r{t}")
                for nt in range(2):
                    nc.scalar.activation(r_sb[:, 512 * nt : 512 * nt + 512],
                                         g["pr"][nt][:], AF.Sigmoid)

                if filler is not None:
                    filler()

                # n = tanh(in + r*hn) per half; transpose blocks as halves land
                n_sb = gsb.tile([64, 1024], F16, tag="n", name=f"n{t}")
                pnT = ps.tile([128, 512], F32, tag="ps", name=f"pnT{t}")
                for nt in range(2):
                    t1 = tmpp.tile([64, 512], F32, tag="t1", name=f"t1_{t}_{nt}")
                    nc.vector.tensor_mul(t1[:], r_sb[:, 512 * nt : 512 * nt + 512],
                                         g["phn"][nt][:])
                    nc.vector.tensor_add(t1[:], t1[:], g["pin"][nt][:])
                    nc.scalar.activation(n_sb[:, 512 * nt : 512 * nt + 512],
                                         t1[:], AF.Tanh)
                    for jh in range(4 * nt, 4 * nt + 4):
                        nc.tensor.matmul(
                            pnT[:, 64 * jh : 64 * jh + 64],
                            n_sb[0:64, 128 * jh : 128 * jh + 128],
                            iden[:, :], start=True, stop=True,
                        )

                # hT' = nT + zT*(hT - nT), pipelined by 256-col halves
                for hh in range(2):
                    s = slice(256 * hh, 256 * hh + 256)
                    d = tmpp.tile([128, 256], F32, tag="d", name=f"d{t}_{hh}")
                    nc.vector.tensor_sub(d[:], hin(hh), pnT[:, s])
                    nc.vector.tensor_mul(d[:], pzT[:, s], d[:])
                    nc.vector.tensor_add(hout(hh), pnT[:, s], d[:])

            # ================= encoder =================
            xt_tiles = {}
            for t in range(min(3, n_steps)):
                xt_tiles[t] = xts.tile([128, 4, BL], F16, tag="x", name=f"xt{t}")
                for k in range(4):
                    nc.sync.dma_start(xt_tiles[t][:, k, :], xT_d[t, k])
            cur = {}
            cur["pz"] = alloc_pair("pz", 0)
            cur["pr"] = alloc_pair("pr", 0)
            emit_gi_zr(cur, xt_tiles[0])
            cur["pin"] = alloc_pair("pin", 0)
            emit_gi_in(cur, xt_tiles[0])
            add_bias(cur, biasE, ("pin",))
            cur["phn"] = alloc_pair("phn", 0)
            hT = hT0
            for t in range(n_steps):
                if t + 3 < n_steps:
                    xt_tiles[t + 3] = xts.tile([128, 4, BL], F16, tag="x",
                                               name=f"xt{t+3}")
                    for k in range(4):
                        nc.sync.dma_start(xt_tiles[t + 3][:, k, :], xT_d[t + 3, k])
                hT_prev = hT
                emit_gh(cur, whh, (("pz", 1024), ("pr", 0), ("phn", 2048)),
                        lambda k: hT_prev[:, 64 * k : 64 * k + 64],
                        starts=("phn",))
                add_bias(cur, biasE, ("pz", "pr", "phn"))
                g = cur
                nxt = {}
                if t + 1 < n_steps:
                    xt_next = xt_tiles[t + 1]

                    def filler(nxt=nxt, xt_next=xt_next, t=t):
                        nxt["pz"] = alloc_pair("pz", t + 1)
                        nxt["pr"] = alloc_pair("pr", t + 1)
                        emit_gi_zr(nxt, xt_next)
                else:
                    filler = None
                hT_new = hst.tile([128, 512], F16, tag="h", name=f"h{t}")
                step_tail(
                    t, g, filler,
                    lambda hh, hT_prev=hT_prev: hT_prev[:, 256 * hh : 256 * hh + 256],
                    lambda hh, hT_new=hT_new: hT_new[:, 256 * hh : 256 * hh + 256],
                )
                hT = hT_new
                if t + 1 < n_steps:
                    nxt["pin"] = alloc_pair("pin", t + 1)
                    emit_gi_in(nxt, xt_tiles[t + 1])
                    add_bias(nxt, biasE, ("pin",))
                    nxt["phn"] = alloc_pair("phn", t + 1)
                cur = nxt
                xt_tiles.pop(t, None)

            # ================= decoder =================
            enc_hT = hT  # [128, 512] final encoder state
            cur = {}
            for nm in ("pz", "pr", "pin", "phn"):
                cur[nm] = alloc_pair(nm, 1000)
            hTd = None       # current pair tile [128, 2, 8, BL]
            hTd_prev = None  # previous pair tile
            pend = None      # (pzo, pair_tile, t0) with k4..7 outstanding
            for t in range(n_steps):
                if t == 0:
                    stat = lambda k: enc_hT[:, 64 * k : 64 * k + 64]
                elif t % 2 == 1:
                    stat = lambda k, _h=hTd: _h[:, k, 0, :]
                else:
                    stat = lambda k, _h=hTd: _h[:, k, 1, :]
                if t % 2 == 0:
                    hTd_prev = hTd
                    hTd = hsd.tile([128, 8, 2, BL], F16, tag="hd", name=f"hd{t}")
                emit_gh(cur, wcb, (("pz", 1024), ("pr", 0), ("pin", 2048),
                                   ("phn", 3072)), stat,
                        starts=("pz", "pr", "pin", "phn"))
                add_bias(cur, biasD, ("pz", "pr", "pin", "phn"))
                g = cur
                nxt = {}

                def filler(nxt=nxt, t=t, zp=hTd_prev, last=(t + 1 >= n_steps)):
                    nonlocal pend
                    if t % 2 == 0 and pend is not None:
                        pzo, zp2, t0 = pend
                        zfill_second(pzo, zp2, t0)
                        pend = None
                    if not last:
                        nxt["pz"] = alloc_pair("pz", 1001 + t)
                        nxt["pr"] = alloc_pair("pr", 1001 + t)
                    if t % 2 == 1 and t >= 3:
                        pend = (zfill_first(zp, t - 3), zp, t - 3)

                hin_t = (
                    (lambda hh, _e=enc_hT: _e[:, 256 * hh : 256 * hh + 256])
                    if t == 0
                    else (lambda hh, _h=hTd if t % 2 == 1 else hTd_prev,
                          _half=(t - 1) % 2:
                          _h[:, 4 * hh : 4 * hh + 4, _half, :])
                )
                hout_t = (lambda hh, _h=hTd, _half=t % 2:
                          _h[:, 4 * hh : 4 * hh + 4, _half, :])
                step_tail(1000 + t, g, filler, hin_t, hout_t)
                if t + 1 < n_steps:
                    nxt["pin"] = alloc_pair("pin", 1001 + t)
                    nxt["phn"] = alloc_pair("phn", 1001 + t)
                cur = nxt
            # flush remaining z pairs: (124,125) second half, then (126,127)
            if pend is not None:
                pzo, zp2, t0 = pend
                zfill_second(pzo, zp2, t0)
            pzo = zfill_first(hTd, n_steps - 2)
            zfill_second(pzo, hTd, n_steps - 2)
    return nc


# ---------------------------------------------------------------- host side
def _prep_shared(enc_Wih, enc_Whh, enc_bih, enc_bhh,
                 dec_Wih, dec_Whh, dec_bih, dec_bhh, Wz, bz):
    f16, f32 = np.float16, np.float32
    wihT = np.ascontiguousarray(enc_Wih.T, dtype=f16).reshape(4, 128, 3 * H)
    whhT = np.ascontiguousarray(enc_Whh.T, dtype=f16).reshape(8, 128, 3 * H)
    wcomb = np.concatenate(
        [dec_Wih[: 2 * H] + dec_Whh[: 2 * H], dec_Wih[2 * H :], dec_Whh[2 * H :]], 0
    )
    wcombT = np.ascontiguousarray(wcomb.T, dtype=f16).reshape(8, 128, 4 * H)
    wzT = np.ascontiguousarray(Wz.T, dtype=f16).reshape(8, 128, I)

    def bias8(bih, bhh):
        rz = np.asarray(bih[: 2 * H] + bhh[: 2 * H], f32)
        rows = np.stack([
            rz[1024:1536], rz[1536:2048],          # z0, z1
            rz[0:512], rz[512:1024],               # r0, r1
            np.asarray(bih[2048:2560], f32), np.asarray(bih[2560:3072], f32),
            np.asarray(bhh[2048:2560], f32), np.asarray(bhh[2560:3072], f32),
        ])  # [8, 512]
        return np.ascontiguousarray(
            np.broadcast_to(rows[:, None, :], (8, 64, 512)), dtype=f16)

    biasE = bias8(enc_bih, enc_bhh)
    biasD = bias8(dec_bih, dec_bhh)
    bzb = np.ascontiguousarray(
        np.broadcast_to(np.asarray(bz, f32)[None, :], (128, I)), dtype=f32)
    iden = np.eye(64, dtype=f16)
    h0T = np.full((128, 512), 0.1, f16)
    return {
        "wihT": wihT, "whhT": whhT, "wcombT": wcombT, "wzT": wzT,
        "biasE": biasE, "biasD": biasD, "bzb": bzb,
        "iden": iden, "h0T": h0T,
    }


def kernel(x, enc_Wih, enc_Whh, enc_bih, enc_bhh,
           dec_Wih, dec_Whh, dec_bih, dec_bhh, Wz, bz, n_steps=T):
    x = np.asarray(x, np.float32)
    shared = _prep_shared(enc_Wih, enc_Whh, enc_bih, enc_bhh,
                          dec_Wih, dec_Whh, dec_bih, dec_bhh, Wz, bz)
    in_maps = []
    for c in range(N_CORES):
        xc = x[c * BL : (c + 1) * BL, :n_steps]  # [BL, n_steps, I]
        xT = np.ascontiguousarray(
            xc.transpose(1, 2, 0), dtype=np.float16).reshape(n_steps, 4, 128, BL)
        in_maps.append({"xT": xT, **shared})

    nc = build_nc(n_steps)
    _split_multi_waits(nc)

    trace = bool(int(os.environ.get("GRU_TRACE", "0")))
    if trace:
        _install_ntff_hook()
    res = bass_utils.run_bass_kernel_spmd(
        nc, in_maps, core_ids=list(range(N_CORES)), trace=trace
    )
    if trace and res.exec_time_ns is not None:
        print(f"HW exec time: {res.exec_time_ns} ns")
    out = np.concatenate([res.results[c]["z"] for c in range(N_CORES)], axis=0)
    return out


# revision 12
# speedup vs baseline: 1.3997x; 1.0145x over previous
"""GRU autoencoder Trainium2 kernel.

Data-parallel over batch: 8 cores x 64 rows. Per core, the recurrence keeps
the hidden state TRANSPOSED in SBUF (hT[klo, 64*khi+b] = h[b, 128*khi+klo])
so it can be the stationary matmul operand directly. Gates are computed as
h @ W.T with fp16 weights (moving operand) accumulating in PSUM; PSUM bias
init is done by Act/DVE copies from precomputed bias tiles instead of K=1
matmul seeds, keeping the PE free for gate matmuls. z/n gates are transposed
back via identity matmuls so the hidden update runs in transposed space.
Decoder z-outputs are computed in step-pairs (M=128 stationary spanning a
double-wide hT tile), with the 8-matmul chain split across two consecutive
step tails so the PE pipe stays full.
"""
import os
import sys
import types

import numpy as np

import concourse.bass as bass
import concourse.mybir as mybir
import concourse.tile as tile
from concourse import bass_utils

F32 = mybir.dt.float32
F16 = mybir.dt.float16
AF = mybir.ActivationFunctionType
OP = mybir.AluOpType

N_CORES = 8
B, T, I, H = 512, 128, 512, 1024
BL = B // N_CORES  # 64


# ---------------------------------------------------------------- fixups
_CTRL_OPCODES = {"Drain", "NoOp", "EventSemaphore", "AllEngineBarrier", "Halt"}


def _split_multi_waits(nc, max_waits=1):
    """This walrus build allows only one sync-wait per instruction; hoist
    excess waits onto preceding NoOps (same engine, so semantics hold)."""
    for f in nc.m.functions:
        for blk in f.blocks:
            insts = blk.instructions
            if not any(
                i.sync_info is not None
                and i.sync_info.on_wait
                and len(i.sync_info.on_wait) > max_waits
                for i in insts
            ):
                continue
            new = []
            for inst in insts:
                si = inst.sync_info
                if si is not None and si.on_wait and len(si.on_wait) > max_waits:
                    waits = list(si.on_wait)
                    extra, keep = waits[:-max_waits], waits[-max_waits:]
                    for cs in range(0, len(extra), max_waits):
                        nop = mybir.InstNoOp(
                            name=nc.get_next_instruction_name(),
                            engine=inst.engine,
                            ins=[],
                            outs=[],
                            sync_info=mybir.SyncInfo(
                                on_wait=extra[cs : cs + max_waits], on_update=[]
                            ),
                        )
                        nc.register_instruction(nop)
                        new.append(nop)
                    si.on_wait = keep
                new.append(inst)
            insts[:] = new


def _install_ntff_hook():
    if "antenv.axon_hooks" in sys.modules:
        return True
    mod = types.ModuleType("antenv.axon_hooks")
    state = {"hook": None}
    mod.set_axon_ntff_profile_hook = lambda h: state.__setitem__("hook", h)
    mod.get_axon_ntff_profile_hook = lambda: state["hook"]
    sys.modules["antenv.axon_hooks"] = mod
    try:
        import antenv

        antenv.axon_hooks = mod
        from trn_agent_boot.trn_boot import _ntff_profile_via_ctypes

        hook = _ntff_profile_via_ctypes("/opt/axon/libaxon_pjrt.so")
        if hook is None:
            return False
        mod.set_axon_ntff_profile_hook(hook)
        return True
    except Exception:
        return False


# ---------------------------------------------------------------- program
def build_nc(n_steps=T):
    nc = bass.Bass("TRN2", target_bir_lowering=False, debug=False, num_devices=N_CORES)

    xT_d = nc.dram_tensor("xT", [n_steps, 4, 128, BL], F16, kind="ExternalInput").ap()
    wih_d = nc.dram_tensor("wihT", [4, 128, 3 * H], F16, kind="ExternalInput").ap()
    whh_d = nc.dram_tensor("whhT", [8, 128, 3 * H], F16, kind="ExternalInput").ap()
    wcb_d = nc.dram_tensor("wcombT", [8, 128, 4 * H], F16, kind="ExternalInput").ap()
    wz_d = nc.dram_tensor("wzT", [8, 128, I], F16, kind="ExternalInput").ap()
    be_d = nc.dram_tensor("biasE", [8, 64, 512], F16, kind="ExternalInput").ap()
    bd_d = nc.dram_tensor("biasD", [8, 64, 512], F16, kind="ExternalInput").ap()
    bzb_d = nc.dram_tensor("bzb", [128, I], F32, kind="ExternalInput").ap()
    id_d = nc.dram_tensor("iden", [64, 64], F16, kind="ExternalInput").ap()
    h0_d = nc.dram_tensor("h0T", [128, 512], F16, kind="ExternalInput").ap()
    z_d = nc.dram_tensor("z", [BL, n_steps, I], F32, kind="ExternalOutput").ap()

    with tile.TileContext(nc) as tc:
        with (
            tc.tile_pool(name="cst", bufs=1) as cst,
            tc.tile_pool(name="hst", bufs=3) as hst,
            tc.tile_pool(name="hsd", bufs=3) as hsd,
            tc.tile_pool(name="xts", bufs=3) as xts,
            tc.tile_pool(name="gsb", bufs=2) as gsb,
            tc.tile_pool(name="tmp", bufs=2) as tmpp,
            tc.tile_pool(name="zo", bufs=2) as zop,
            tc.tile_pool(name="ps", bufs=8, space="PSUM") as ps,
        ):
            biasE = cst.tile([64, 8, 512], F16)
            for j in range(8):
                nc.sync.dma_start(biasE[:, j, :], be_d[j])
            biasD = cst.tile([64, 8, 512], F16)
            for j in range(8):
                nc.sync.dma_start(biasD[:, j, :], bd_d[j])
            bzb = cst.tile([128, I], F32)
            nc.sync.dma_start(bzb[:], bzb_d[:])
            iden = cst.tile([64, 64], F16)
            nc.sync.dma_start(iden[:], id_d[:])
            hT0 = hst.tile([128, 512], F16, tag="h")
            nc.sync.dma_start(hT0[:], h0_d[:])
            # all weights resident in fp16 (18MB total)
            wih = cst.tile([128, 4, 3 * H], F16)
            for k in range(4):
                nc.sync.dma_start(wih[:, k, :], wih_d[k])
            whh = cst.tile([128, 8, 3 * H], F16)
            for k in range(8):
                nc.sync.dma_start(whh[:, k, :], whh_d[k])
            wcb = cst.tile([128, 8, 4 * H], F16)
            for k in range(8):
                nc.sync.dma_start(wcb[:, k, :], wcb_d[k])
            wz = cst.tile([128, 8, I], F16)
            for k in range(8):
                nc.sync.dma_start(wz[:, k, :], wz_d[k])

            BIAS_J = {"pz": 0, "pr": 2, "pin": 4, "phn": 6}

            def alloc_pair(nm, t):
                """Allocate one gate pair (2 psum tiles)."""
                return [ps.tile([64, 512], F32, tag="ps", name=f"{nm}{i}_{t}")
                        for i in range(2)]

            def add_bias(g, bias, names):
                for nm in names:
                    j0 = BIAS_J[nm]
                    for nt in range(2):
                        nc.vector.tensor_add(g[nm][nt][:], g[nm][nt][:],
                                             bias[:, j0 + nt, :])

            def emit_gi_zr(g, xt):
                for tiles, c0 in ((g["pz"], 1024), (g["pr"], 0)):
                    for nt in range(2):
                        c = c0 + 512 * nt
                        for k in range(4):
                            nc.tensor.matmul(
                                tiles[nt][:], xt[:, k, :], wih[:, k, c : c + 512],
                                start=(k == 0), stop=False, skip_group_check=True,
                            )

            def emit_gi_in(g, xt):
                for nt in range(2):
                    c = 2048 + 512 * nt
                    for k in range(4):
                        nc.tensor.matmul(
                            g["pin"][nt][:], xt[:, k, :], wih[:, k, c : c + 512],
                            start=(k == 0), stop=(k == 3), skip_group_check=True,
                        )

            def emit_gh(g, w, cols, stat, starts=()):
                """Recurrent gate matmuls; stat(k) -> [128,64] stationary AP."""
                for nm, c0 in cols:
                    for nt in range(2):
                        c = c0 + 512 * nt
                        for k in range(8):
                            nc.tensor.matmul(
                                g[nm][nt][:], stat(k), w[:, k, c : c + 512],
                                start=(k == 0 and nm in starts),
                                stop=(k == 7), skip_group_check=True,
                            )

            def zfill_first(hTd_pair, t0):
                """z-output pair (t0, t0+1): bias init + first 4 k-chunks."""
                pzo = ps.tile([128, 512], F32, tag="ps", name=f"pzo{t0}")
                for j in range(4):
                    nc.tensor.matmul(
                        pzo[:], hTd_pair[:, j, :, :], wz[:, j, :],
                        start=(j == 0), stop=False, skip_group_check=True,
                    )
                return pzo

            def zfill_second(pzo, hTd_pair, t0):
                """z-output pair (t0, t0+1): last 4 k-chunks + writeback."""
                for j in range(4, 8):
                    nc.tensor.matmul(
                        pzo[:], hTd_pair[:, j, :, :], wz[:, j, :],
                        start=False, stop=(j == 7), skip_group_check=True,
                    )
                zo_sb = zop.tile([128, 512], F32, tag="zo", name=f"zo{t0}")
                nc.vector.tensor_add(zo_sb[:], pzo[:], bzb[:])
                nc.sync.dma_start(z_d[:, t0, :], zo_sb[0:64, :])
                nc.sync.dma_start(z_d[:, t0 + 1, :], zo_sb[64:128, :])

            def step_tail(t, g, filler, hin, hout):
                """sigmoids, transposes, n-chain, h-update; filler() emits
                next-step PE work between zT and nT transposes.
                hin(hh)/hout(hh) -> [128, 256]-sized APs for half hh."""
                z_sb = gsb.tile([64, 1024], F16, tag="z", name=f"z{t}")
                for nt in range(2):
                    nc.scalar.activation(z_sb[:, 512 * nt : 512 * nt + 512],
                                         g["pz"][nt][:], AF.Sigmoid)
                pzT = ps.tile([128, 512], F32, tag="ps", name=f"pzT{t}")
                for jh in range(8):
                    nc.tensor.matmul(
                        pzT[:, 64 * jh : 64 * jh + 64],
                        z_sb[0:64, 128 * jh : 128 * jh + 128],
                        iden[:, :], start=True, stop=True,
                    )
                r_sb = gsb.tile([64, 1024], F16, tag="r", name=f"r{t}")
                for nt in range(2):
                    nc.scalar.activation(r_sb[:, 512 * nt : 512 * nt + 512],
                                         g["pr"][nt][:], AF.Sigmoid)

                if filler is not None:
                    filler()

                # n = tanh(in + r*hn) per half; transpose blocks as halves land
                n_sb = gsb.tile([64, 1024], F16, tag="n", name=f"n{t}")
                pnT = ps.tile([128, 512], F32, tag="ps", name=f"pnT{t}")
                def upd(hh):
                    # hT' = nT + zT*(hT - nT) for one 256-col half
                    s = slice(256 * hh, 256 * hh + 256)
                    d = tmpp.tile([128, 256], F32, tag="d", name=f"d{t}_{hh}")
                    nc.vector.tensor_sub(d[:], hin(hh), pnT[:, s])
                    nc.vector.tensor_mul(d[:], pzT[:, s], d[:])
                    nc.vector.tensor_add(hout(hh), pnT[:, s], d[:])

                for nt in range(2):
                    t1 = tmpp.tile([64, 512], F32, tag="t1", name=f"t1_{t}_{nt}")
                    nc.vector.tensor_mul(t1[:], r_sb[:, 512 * nt : 512 * nt + 512],
                                         g["phn"][nt][:])
                    nc.vector.tensor_add(t1[:], t1[:], g["pin"][nt][:])
                    nc.scalar.activation(n_sb[:, 512 * nt : 512 * nt + 512],
                                         t1[:], AF.Tanh)
                    for jh in range(4 * nt, 4 * nt + 4):
                        nc.tensor.matmul(
                            pnT[:, 64 * jh : 64 * jh + 64],
                            n_sb[0:64, 128 * jh : 128 * jh + 128],
                            iden[:, :], start=True, stop=True,
                        )
                    upd(nt)

            # ================= encoder =================
            xt_tiles = {}
            for t in range(min(3, n_steps)):
                xt_tiles[t] = xts.tile([128, 4, BL], F16, tag="x", name=f"xt{t}")
                for k in range(4):
                    nc.sync.dma_start(xt_tiles[t][:, k, :], xT_d[t, k])
            cur = {}
            cur["pz"] = alloc_pair("pz", 0)
            cur["pr"] = alloc_pair("pr", 0)
            emit_gi_zr(cur, xt_tiles[0])
            cur["pin"] = alloc_pair("pin", 0)
            emit_gi_in(cur, xt_tiles[0])
            add_bias(cur, biasE, ("pin",))
            cur["phn"] = alloc_pair("phn", 0)
            hT = hT0
            for t in range(n_steps):
                if t + 3 < n_steps:
                    xt_tiles[t + 3] = xts.tile([128, 4, BL], F16, tag="x",
                                               name=f"xt{t+3}")
                    for k in range(4):
                        nc.sync.dma_start(xt_tiles[t + 3][:, k, :], xT_d[t + 3, k])
                hT_prev = hT
                emit_gh(cur, whh, (("pz", 1024), ("pr", 0), ("phn", 2048)),
                        lambda k: hT_prev[:, 64 * k : 64 * k + 64],
                        starts=("phn",))
                add_bias(cur, biasE, ("pz", "pr", "phn"))
                g = cur
                nxt = {}
                if t + 1 < n_steps:
                    xt_next = xt_tiles[t + 1]

                    def filler(nxt=nxt, xt_next=xt_next, t=t):
                        nxt["pz"] = alloc_pair("pz", t + 1)
                        nxt["pr"] = alloc_pair("pr", t + 1)
                        emit_gi_zr(nxt, xt_next)
                else:
                    filler = None
                hT_new = hst.tile([128, 512], F16, tag="h", name=f"h{t}")
                step_tail(
                    t, g, filler,
                    lambda hh, hT_prev=hT_prev: hT_prev[:, 256 * hh : 256 * hh + 256],
                    lambda hh, hT_new=hT_new: hT_new[:, 256 * hh : 256 * hh + 256],
                )
                hT = hT_new
                if t + 1 < n_steps:
                    nxt["pin"] = alloc_pair("pin", t + 1)
                    emit_gi_in(nxt, xt_tiles[t + 1])
                    add_bias(nxt, biasE, ("pin",))
                    nxt["phn"] = alloc_pair("phn", t + 1)
                cur = nxt
                xt_tiles.pop(t, None)

            # ================= decoder =================
            enc_hT = hT  # [128, 512] final encoder state
            cur = {}
            for nm in ("pz", "pr", "pin", "phn"):
                cur[nm] = alloc_pair(nm, 1000)
            hTd = None       # current pair tile [128, 2, 8, BL]
            hTd_prev = None  # previous pair tile
            pend = None      # (pzo, pair_tile, t0) with k4..7 outstanding
            for t in range(n_steps):
                if t == 0:
                    stat = lambda k: enc_hT[:, 64 * k : 64 * k + 64]
                elif t % 2 == 1:
                    stat = lambda k, _h=hTd: _h[:, k, 0, :]
                else:
                    stat = lambda k, _h=hTd: _h[:, k, 1, :]
                if t % 2 == 0:
                    hTd_prev = hTd
                    hTd = hsd.tile([128, 8, 2, BL], F16, tag="hd", name=f"hd{t}")
                emit_gh(cur, wcb, (("pz", 1024), ("pr", 0), ("pin", 2048),
                                   ("phn", 3072)), stat,
                        starts=("pz", "pr", "pin", "phn"))
                add_bias(cur, biasD, ("pz", "pr", "pin", "phn"))
                g = cur
                nxt = {}

                def filler(nxt=nxt, t=t, zp=hTd_prev, last=(t + 1 >= n_steps)):
                    nonlocal pend
                    if t % 2 == 0 and pend is not None:
                        pzo, zp2, t0 = pend
                        zfill_second(pzo, zp2, t0)
                        pend = None
                    if not last:
                        nxt["pz"] = alloc_pair("pz", 1001 + t)
                        nxt["pr"] = alloc_pair("pr", 1001 + t)
                    if t % 2 == 1 and t >= 3:
                        pend = (zfill_first(zp, t - 3), zp, t - 3)

                hin_t = (
                    (lambda hh, _e=enc_hT: _e[:, 256 * hh : 256 * hh + 256])
                    if t == 0
                    else (lambda hh, _h=hTd if t % 2 == 1 else hTd_prev,
                          _half=(t - 1) % 2:
                          _h[:, 4 * hh : 4 * hh + 4, _half, :])
                )
                hout_t = (lambda hh, _h=hTd, _half=t % 2:
                          _h[:, 4 * hh : 4 * hh + 4, _half, :])
                step_tail(1000 + t, g, filler, hin_t, hout_t)
                if t + 1 < n_steps:
                    nxt["pin"] = alloc_pair("pin", 1001 + t)
                    nxt["phn"] = alloc_pair("phn", 1001 + t)
                cur = nxt
            # flush remaining z pairs: (124,125) second half, then (126,127)
            if pend is not None:
                pzo, zp2, t0 = pend
                zfill_second(pzo, zp2, t0)
            pzo = zfill_first(hTd, n_steps - 2)
            zfill_second(pzo, hTd, n_steps - 2)
    return nc


# ---------------------------------------------------------------- host side
def _prep_shared(enc_Wih, enc_Whh, enc_bih, enc_bhh,
                 dec_Wih, dec_Whh, dec_bih, dec_bhh, Wz, bz):
    f16, f32 = np.float16, np.float32
    wihT = np.ascontiguousarray(enc_Wih.T, dtype=f16).reshape(4, 128, 3 * H)
    whhT = np.ascontiguousarray(enc_Whh.T, dtype=f16).reshape(8, 128, 3 * H)
    wcomb = np.concatenate(
        [dec_Wih[: 2 * H] + dec_Whh[: 2 * H], dec_Wih[2 * H :], dec_Whh[2 * H :]], 0
    )
    wcombT = np.ascontiguousarray(wcomb.T, dtype=f16).reshape(8, 128, 4 * H)
    wzT = np.ascontiguousarray(Wz.T, dtype=f16).reshape(8, 128, I)

    def bias8(bih, bhh):
        rz = np.asarray(bih[: 2 * H] + bhh[: 2 * H], f32)
        rows = np.stack([
            rz[1024:1536], rz[1536:2048],          # z0, z1
            rz[0:512], rz[512:1024],               # r0, r1
            np.asarray(bih[2048:2560], f32), np.asarray(bih[2560:3072], f32),
            np.asarray(bhh[2048:2560], f32), np.asarray(bhh[2560:3072], f32),
        ])  # [8, 512]
        return np.ascontiguousarray(
            np.broadcast_to(rows[:, None, :], (8, 64, 512)), dtype=f16)

    biasE = bias8(enc_bih, enc_bhh)
    biasD = bias8(dec_bih, dec_bhh)
    bzb = np.ascontiguousarray(
        np.broadcast_to(np.asarray(bz, f32)[None, :], (128, I)), dtype=f32)
    iden = np.eye(64, dtype=f16)
    h0T = np.full((128, 512), 0.1, f16)
    return {
        "wihT": wihT, "whhT": whhT, "wcombT": wcombT, "wzT": wzT,
        "biasE": biasE, "biasD": biasD, "bzb": bzb,
        "iden": iden, "h0T": h0T,
    }


def kernel(x, enc_Wih, enc_Whh, enc_bih, enc_bhh,
           dec_Wih, dec_Whh, dec_bih, dec_bhh, Wz, bz, n_steps=T):
    x = np.asarray(x, np.float32)
    shared = _prep_shared(enc_Wih, enc_Whh, enc_bih, enc_bhh,
                          dec_Wih, dec_Whh, dec_bih, dec_bhh, Wz, bz)
    in_maps = []
    for c in range(N_CORES):
        xc = x[c * BL : (c + 1) * BL, :n_steps]  # [BL, n_steps, I]
        xT = np.ascontiguousarray(
            xc.transpose(1, 2, 0), dtype=np.float16).reshape(n_steps, 4, 128, BL)
        in_maps.append({"xT": xT, **shared})

    nc = build_nc(n_steps)
    _split_multi_waits(nc)

    trace = bool(int(os.environ.get("GRU_TRACE", "0")))
    if trace:
        _install_ntff_hook()
    res = bass_utils.run_bass_kernel_spmd(
        nc, in_maps, core_ids=list(range(N_CORES)), trace=trace
    )
    if trace and res.exec_time_ns is not None:
        print(f"HW exec time: {res.exec_time_ns} ns")
    out = np.concatenate([res.results[c]["z"] for c in range(N_CORES)], axis=0)
    return out
